# revision 30
# baseline (speedup 1.0000x reference)
"""MixtureLinear Trainium2 kernel.

Computes, for B=256, IN=1024, OUT=1024, RANK=16:
    out[b,o] = sum_i input[b,i] * sum_r weight[o,i,r] * coef[b,r]
             + sum_r bias[o,r] * coef[b,r]

Strategy (8 NeuronCores, tensor-parallel on OUT):
  - Core c owns OUT rows [128c, 128c+128). It reads only its weight shard
    (1/8 of the 64MB weight tensor), input/coef replicated.
  - Stage 1 (PE): proj[b,(o,r)] = inputT.T @ W2 where W2[i, o*16+r] =
    weight[o,i,r]; K=IN accumulated over 8 psum matmuls per 512-column
    chunk (one psum bank, 32 o's x 16 r's per chunk).
  - Stage 2 (DVE): out[b,o] = sum_r proj[b,(o,r)] * coef[b,r] via a
    broadcast-AP multiply + strided reduce over the innermost rank axis.
  - Bias: one tiny K=16 matmul per b-chunk: coefT.T @ biasT -> psum,
    added in the final DVE add before the output DMA.

Matmul dtype is selectable via MIXL_DT (float16 default; bfloat16 /
float32r / float32 supported). Host pre-casts and pre-transposes shards;
stage-2 and all accumulation stay fp32.
"""

import os
import sys
from contextlib import ExitStack

sys.path.insert(0, "/opt/trn_rl_repo")

import numpy as np
import ml_dtypes

import concourse.bass as bass
import concourse.tile as tile
from concourse import bacc, mybir
from concourse.bass_utils import run_bass_kernel_spmd

B, IN, OUT, RANK = 256, 1024, 1024, 16
NCORES = 8
OUTL = OUT // NCORES        # 128 out rows per core
P = 128                     # partitions
NB = B // P                 # 2 batch chunks
NK = IN // P                # 8 contraction chunks
CH = 512                    # psum chunk: one fp32 bank
NCH = OUTL * RANK // CH     # 4 column chunks per core
OCH = CH // RANK            # 32 o's per chunk

DT_NAME = os.environ.get("MIXL_DT", "float16")
IMPL = os.environ.get("MIXL_IMPL", "raw2")

_DT_MAP = {
    "float16": (mybir.dt.float16, np.float16),
    "bfloat16": (mybir.dt.bfloat16, ml_dtypes.bfloat16),
    "float32r": (mybir.dt.float32r, np.float32),
    "float32": (mybir.dt.float32, np.float32),
}


class _NoBarrierBlock(bass.BassBlock):
    """BassBlock without the exit drain + all-engine barrier.

    The NRT epilogue (per-engine semaphore-zero storm + exit rendezvous
    chain, ~7us total) runs after each engine's stream ends. With the
    stock barrier, every engine waits for the slowest one before starting
    its epilogue share; without it, early-finishing engines overlap their
    epilogue with the critical-path tail. Output completion is still
    guaranteed: gpsimd's terminal s_out wait orders NEFF completion after
    the output DMAs.
    """

    def __exit__(self, exc_type, exc_val, exc_tb):
        if exc_type is not None:
            return
        for engine, last_body in self.last_body.items():
            with self.bass.body(
                last_body, parent=self.bass.cur_bb, allow_existing_parent=True
            ):
                engine.br(self.end_bb)
        self.bass.switch_bb(self.end_bb)


def build_nc_raw(dt_name=DT_NAME):
    """Raw-Bass (manual Block + semaphores) implementation.

    Design notes (all HW-measured on this container):
    - The NRT execution envelope (entry rendezvous chain + per-engine
      instruction-table loads at the front; per-engine semaphore-zero
      storm + exit rendezvous at the back) costs ~14us on an empty
      kernel. The exit part runs after each ENGINE's stream ends, so the
      Block-end all-engine barrier is deliberately skipped (engines that
      finish early overlap their epilogue with the critical-path tail).
    - A single DMA transfer only sustains ~100-200 GB/s; aggregate tops
      out ~430 GB/s, and the SDMA engines round-robin across everything
      queued at packet granularity. So loads go out in consumption order
      as ~10 chunks with a sliding window of 3 in flight, with tiny
      first chunks so the first matmul can start ASAP.
    - 8 dummy matmuls on memset data warm the PE HAM clock (1.2->2.4GHz
      after ~3.4us of sustained activity) while the first loads land.
    - LDWEIGHTS is emitted separately from a non-self-loading Matmult
      (inst.ldweights=False) so weight loads pipeline into the PE's
      background buffer; fused matmuls measured ~600ns vs ~380ns split.
    """
    dt, _ = _DT_MAP[dt_name]
    f32 = mybir.dt.float32
    f16 = mybir.dt.float16
    nc = bacc.Bacc("TRN2", target_bir_lowering=False, debug=False)

    xT = nc.declare_dram_parameter("xT", [IN, B], dt, isOutput=False)
    w2 = nc.declare_dram_parameter("w2", [NCH, P, NK * CH], dt, isOutput=False)
    coef = nc.declare_dram_parameter("coef", [B, RANK], f32, isOutput=False)
    coefT = nc.declare_dram_parameter("coefT", [RANK, B], dt, isOutput=False)
    biasT = nc.declare_dram_parameter("biasT", [RANK, OUTL], dt, isOutput=False)
    out = nc.declare_dram_parameter("out", [B, OUTL], f32, isOutput=True)

    w2v = w2.rearrange("n p (k c) -> n p k c", c=CH)
    xTv = xT.rearrange("(k p) b -> p k b", p=P)
    coefv = coef.rearrange("(nb p) r -> p nb r", p=P)

    with ExitStack() as ctx:
        sb = lambda shape, d, name: ctx.enter_context(
            nc.sbuf_tensor(name, shape, d))
        xT_t = sb([P, NK, B], dt, "xT_t")
        wts = [sb([P, NK, CH], dt, f"wt{n}") for n in range(NCH)]
        coef_t = sb([P, NB, RANK], f32, "coef_t")
        coefT_t = sb([RANK, B], dt, "coefT_t")
        biasT_t = sb([RANK, OUTL], dt, "biasT_t")
        warm_t = sb([P, CH], dt, "warm_t")
        tmps = [sb([P, OCH, RANK], f16, f"tmp{i}") for i in range(2)]
        out_sb = [sb([P, OUTL], f32, f"osum{b}") for b in range(NB)]
        outf = [sb([P, OUTL], f32, f"outf{b}") for b in range(NB)]
        pss = [ctx.enter_context(nc.psum_tensor(f"ps{g}", [P, CH], f32))
               for g in range(8)]

        # One semaphore per DMA: +16 increments from different transfers
        # interleave (per-SDMA-engine +1s), so aggregate thresholds on a
        # shared sem do not prove any single transfer completed.
        nsem = lambda name: ctx.enter_context(nc.semaphore(name))
        s_x0 = nsem("s_x0")        # xT k=0 (64KB)
        s_w00 = nsem("s_w00")      # w n0 k=0 (128KB)
        s_x13 = nsem("s_x13")      # xT k=1..3
        s_w013 = nsem("s_w013")    # w n0 k=1..3
        s_x47 = nsem("s_x47")      # xT k=4..7
        s_w047 = nsem("s_w047")    # w n0 k=4..7
        s_wn = [nsem(f"s_wn{n}") for n in range(1, NCH)]   # w1..w3 (1MB)
        s_gc = nsem("s_gc")        # coefT
        s_gb = nsem("s_gb")        # biasT
        s_gf = nsem("s_gf")        # coef (fp32)
        s_warm = nsem("s_warm")    # warm-up tile memset
        s_pe = nsem("s_pe")        # psum groups done
        s_dvm = nsem("s_dvm")      # psum mults done
        s_red = nsem("s_red")      # reduces done
        s_dve = nsem("s_dve")      # outf ready
        s_out = nsem("s_out")      # output DMA done

        with _NoBarrierBlock(nc, f"block_{nc.next_id()}") as block:

            @block.sync
            def _(sync):
                xfers = [
                    (xT_t[:, 0:1, :], xTv[:, 0:1, :], s_x0),
                    (wts[0][:, 0:1, :], w2v[0][:, 0:1, :], s_w00),
                    (xT_t[:, 1:4, :], xTv[:, 1:4, :], s_x13),
                    (wts[0][:, 1:4, :], w2v[0][:, 1:4, :], s_w013),
                    (xT_t[:, 4:, :], xTv[:, 4:, :], s_x47),
                    (wts[0][:, 4:, :], w2v[0][:, 4:, :], s_w047),
                ] + [(wts[n][:], w2v[n], s_wn[n - 1]) for n in range(1, NCH)]
                for i, (dst, srcv, sem) in enumerate(xfers):
                    if i >= 3:
                        sync.wait_ge(xfers[i - 3][2], 16)
                    sync.dma_start(dst, srcv).then_inc(sem, 16)

            @block.scalar
            def _(scalar):
                # Output DMAs on the (otherwise idle) ACT ring.
                for b in range(NB):
                    scalar.wait_ge(s_dve, b + 1)
                    scalar.dma_start(out[b * P:(b + 1) * P, :],
                                     outf[b][:]).then_inc(s_out, 16)

            @block.gpsimd
            def _(gpsimd):
                gpsimd.memset(warm_t[:], 0.25).then_inc(s_warm, 1)
                gpsimd.dma_start(coef_t[:], coefv).then_inc(s_gf, 16)
                gpsimd.dma_start(coefT_t[:], coefT[:]).then_inc(s_gc, 16)
                gpsimd.dma_start(biasT_t[:], biasT[:]).then_inc(s_gb, 16)
                # Terminal waiter: holds the Pool stream until outputs are
                # in DRAM, so NEFF completion implies outputs landed.
                gpsimd.wait_ge(s_out, 32)

            @block.tensor
            def _(pe):
                # HAM warm-up: ~8 dummy matmuls on memset data while the
                # first real chunks are still in flight.
                pe.wait_ge(s_warm, 1)
                for _i in range(8):
                    nc.tensor.matmul(pss[2][:], lhsT=warm_t[:, 0:P],
                                     rhs=warm_t[:], start=True, stop=True)
                for n in range(NCH):
                    bank = (2 * n) % 6
                    for k in range(NK):
                        if n == 0:
                            if k == 0:
                                pe.wait_ge(s_x0, 16)
                                pe.wait_ge(s_w00, 16)
                            elif k == 1:
                                pe.wait_ge(s_x13, 16)
                                pe.wait_ge(s_w013, 16)
                            elif k == 4:
                                pe.wait_ge(s_x47, 16)
                                pe.wait_ge(s_w047, 16)
                        elif k == 0:
                            pe.wait_ge(s_wn[n - 1], 16)
                        if n == 3 and k == 0:
                            # banks 0/1 reused: n0 multiplies must be done
                            pe.wait_ge(s_dvm, 2)
                        for b in range(NB):
                            # split LDWEIGHTS + non-self-loading matmul
                            nc.tensor.ldweights(xT_t[:, k, b * P:(b + 1) * P])
                            mm = nc.tensor.matmul(
                                pss[bank + b][:],
                                lhsT=xT_t[:, k, b * P:(b + 1) * P],
                                rhs=wts[n][:, k, :],
                                start=(k == 0),
                                stop=(k == NK - 1),
                            )
                            mm.ins.ldweights = False
                            if k == NK - 1:
                                mm.then_inc(s_pe, 1)
                    if n == 0:
                        # Bias matmuls into dedicated banks 6/7, slotted here
                        # so their input DMAs are long done and the PE stream
                        # never stalls on them.
                        pe.wait_ge(s_gc, 16)
                        pe.wait_ge(s_gb, 16)
                        for b in range(NB):
                            nc.tensor.matmul(
                                pss[6 + b][:, 0:OUTL],
                                lhsT=coefT_t[:, b * P:(b + 1) * P],
                                rhs=biasT_t[:],
                                start=True, stop=True,
                            ).then_inc(s_pe, 1)

            @block.vector
            def _(vector):
                # Clear every sem except s_warm (gpsimd owns that one so
                # its iota inc is ordered after the clear). Vector is idle
                # early; this completes well before the first DMA
                # completions (~11us) or engine incs can land.
                rest = [s_xa, s_gf, s_gc, s_gb, s_pe, s_pc, s_dvm,
                        s_dve, s_out] + s_wk
                lo = min(s.num for s in rest)
                hi = max(s.num for s in rest)
                vector.sem_clear(range(lo, hi + 1))
                vector.wait_ge(s_gf, 16)
                # s_pe increment order: n0b0=1 n0b1=2 bias0=3 bias1=4
                # n1b0=5 n1b1=6 n2b0=7 n2b1=8 n3b0=9 n3b1=10
                pe_val = {0: (1, 2), 1: (5, 6), 2: (7, 8), 3: (9, 10)}
                g = 0
                for n in range(NCH):
                    bank = (2 * n) % 6
                    for b in range(NB):
                        g += 1
                        vector.wait_ge(s_pe, pe_val[n][b])
                        coef_b = coef_t[:, b, :].rearrange(
                            "p (one r) -> p one r", one=1)
                        tmp = tmps[g % 2]
                        nc.vector.tensor_mul(
                            tmp[:],
                            pss[bank + b][:].rearrange("p (o r) -> p o r", r=RANK),
                            coef_b.to_broadcast((P, OCH, RANK)),
                        ).then_inc(s_dvm, 1)
                        vector.wait_ge(s_dvm, g)
                        nc.vector.tensor_reduce(
                            out_sb[b][:, n * OCH:(n + 1) * OCH],
                            tmp[:],
                            axis=mybir.AxisListType.X,
                            op=mybir.AluOpType.add,
                        ).then_inc(s_red, 1)
                for b in range(NB):
                    vector.wait_ge(s_pe, 3 + b)
                    # all four reduces of this b-chunk (g = b+1, b+3, b+5, b+7)
                    vector.wait_ge(s_red, NB * NCH - NB + b + 1)
                    nc.vector.tensor_add(
                        outf[b][:], out_sb[b][:], pss[6 + b][:, 0:OUTL]
                    ).then_inc(s_dve, 1)

    nc.compile()
    return nc


def build_nc_raw2(dt_name=DT_NAME):
    """v4 raw-Bass implementation. Trace-driven design (see transcript):

    - exec_time = last_useful - first_useful. Fixed ~8.3us envelope inside
      exec (preamble barrier era + ~7.7us per-engine exit epilogue).
    - DMA queues sustain ~425 GB/s only with multi-KB descriptors (one per
      partition row); every DRAM tensor is host-swizzled so each partition
      row is one contiguous multi-KB run.
    - Queues round-robin across ALL doorbelled transfers at descriptor
      granularity, so a later transfer dilutes earlier ones. Weight stream
      is a sem-gated chain: each dma_start is gated on the previous
      transfer reaching GATE/16 increments. GATE is small (~3): doorbell
      +descriptor-fetch is ~1.5us, so firing early lets the next stream
      start right as the previous ends, with only mild tail dilution.
    - PE waits on FULL transfer completion, so weight chunks are split
      into k0-3/k4-7 halves (512KB) to halve completion-wait granularity.
    - PE HAM clock (1.2->2.4GHz) needs ~3.1us of sustained high-toggle
      matmul activity; gaps >0.9us reset it. Warmup runs 256-col matmuls
      on iota data until the first real weights land; small fillers
      bridge later waits.
    - n3 runs b1 before b0 and splits each accumulation into two 256-col
      groups so stage-2 (DVE) and the first output DMA overlap the last
      matmuls (shorter serial tail).
    """
    dt, _ = _DT_MAP[dt_name]
    f32 = mybir.dt.float32
    f16 = mybir.dt.float16
    nc = bacc.Bacc("TRN2", target_bir_lowering=False, debug=False)

    xT2 = nc.declare_dram_parameter("xT2", [P, NK * B], dt, isOutput=False)
    w2 = nc.declare_dram_parameter("w2", [NCH, P, NK * CH], dt, isOutput=False)
    coef2 = nc.declare_dram_parameter("coef2", [P, NB * RANK], f32, isOutput=False)
    coefT = nc.declare_dram_parameter("coefT", [RANK, B], dt, isOutput=False)
    biasT = nc.declare_dram_parameter("biasT", [RANK, OUTL], dt, isOutput=False)
    out = nc.declare_dram_parameter("out", [B, OUTL], f32, isOutput=True)

    w2v = w2.rearrange("n p (k c) -> n p k c", c=CH)
    xv = xT2.rearrange("p (k b) -> p k b", b=B)
    cv = coef2.rearrange("p (nb r) -> p nb r", r=RANK)

    NWARM = int(os.environ.get("MIXL_NWARM", "11"))
    FILLS = [int(x) for x in os.environ.get("MIXL_FILLS", "0,0,0").split(",")]
    N3SPLIT = os.environ.get("MIXL_N3SPLIT", "0") == "1"
    WARMC = int(os.environ.get("MIXL_WARMC", "512"))

    with ExitStack() as ctx:
        sb = lambda shape, d, name: ctx.enter_context(
            nc.sbuf_tensor(name, shape, d))
        xT_t = sb([P, NK, B], dt, "xT_t")
        wts = [sb([P, NK, CH], dt, f"wt{n}") for n in range(NCH)]
        coef_t = sb([P, NB, RANK], f32, "coef_t")
        coefT_t = sb([RANK, B], dt, "coefT_t")
        biasT_t = sb([RANK, OUTL], dt, "biasT_t")
        warm_t = sb([P, 4 * P], f16, "warm_t")
        tmp = sb([P, OCH, RANK], f16, "tmp")
        tmph = sb([P, OCH // 2, RANK], f16, "tmph")
        out_sb = [sb([P, OUTL], f32, f"osum{b}") for b in range(NB)]
        outf = [sb([P, OUTL], f32, f"outf{b}") for b in range(NB)]
        pss = [ctx.enter_context(nc.psum_tensor(f"ps{g}", [P, CH], f32))
               for g in range(8)]

        nsem = lambda name: ctx.enter_context(nc.semaphore(name))
        s_warm = nsem("s_warm")
        s_xa = nsem("s_xa")        # xT2 full (512KB, 4KB rows)
        s_wk = [nsem(f"s_wk{i}") for i in range(5)]
        s_gf = nsem("s_gf")
        s_gc = nsem("s_gc")
        s_gb = nsem("s_gb")
        s_pe = nsem("s_pe")
        s_pc = nsem("s_pc")        # mid-chunk pacing for w2/w3 doorbells
        s_dvm = nsem("s_dvm")
        s_dve = nsem("s_dve")
        s_out = nsem("s_out")

        # psum banks: n0 -> 0/1, n1 -> 2/3, n2 -> 4/5, n3 -> 0/1 (after
        # vector consumed n0), bias b0/b1 -> bank 6 cols 0/128, warm -> 7.
        BANK = {0: 0, 1: 2, 2: 4, 3: 0}
        # s_pe increment order:
        # n0b0=1 n0b1=2 n1b0=3 n1b1=4 bias0=5 bias1=6 n2b0=7 n2b1=8
        # n3b1A=9 n3b1B=10 n3b0A=11 n3b0B=12

        with _NoBarrierBlock(nc, f"block_{nc.next_id()}") as block:

            @block.sync
            def _(sync):
                # Early chain upfront (round-robin dilution among these
                # four is acceptable; completion tracks doorbell order).
                # w2/w3 are paced by PE progress: issuing them when the PE
                # proves it is one chunk away keeps w1's stream undiluted
                # without sem-gated holes (all 16 completion increments of
                # a transfer arrive in its last ~0.7us, so gating on the
                # PREVIOUS transfer can never hide the ~1.5us doorbell+
                # descriptor-fetch latency).
                # Single-ring FIFO: each engine's DGE ring drains its
                # transfers in issue order (measured: back-to-back sync
                # transfers complete sequentially at ~420GB/s), while
                # DIFFERENT rings fair-share per descriptor slot. So the
                # entire consumption-ordered chain goes on one ring with
                # no gating; only the tiny trio rides another ring.
                sync.dma_start(xT_t[:], xv[:]).then_inc(s_xa, 16)
                sync.dma_start(wts[0][:, 0:4, :],
                               w2v[0][:, 0:4, :]).then_inc(s_wk[0], 16)
                sync.dma_start(wts[0][:, 4:, :],
                               w2v[0][:, 4:, :]).then_inc(s_wk[4], 16)
                sync.dma_start(wts[1][:], w2v[1]).then_inc(s_wk[1], 16)
                sync.dma_start(wts[2][:], w2v[2]).then_inc(s_wk[2], 16)
                sync.dma_start(wts[3][:], w2v[3]).then_inc(s_wk[3], 16)

            @block.scalar
            def _(scalar):
                # b1 finishes first (n3 computes b1 before b0)
                for i, b in enumerate((1, 0)):
                    scalar.wait_ge(s_dve, i + 1)
                    scalar.dma_start(out[b * P:(b + 1) * P, :],
                                     outf[b][:]).then_inc(s_out, 16)

            @block.gpsimd
            def _(gpsimd):
                # Stale-state hygiene: a prior execution of this NEFF (or a
                # prior NEFF) can leave nonzero values in our sem range; a
                # leftover +1 makes every wait_ge(sem, 16) fire one DMA
                # batch early (observed: s_xa reached 17, PE read xT mid-
                # flight -> NaN). Clear before anything can increment:
                # gpsimd runs this before its iota (s_warm inc), and the
                # first DMA completions land several us later.
                gpsimd.sem_clear(range(s_warm.num, s_warm.num + 1))
                gpsimd.iota(warm_t[:], pattern=[[1, 4 * P]], base=0,
                            channel_multiplier=63,
                            allow_small_or_imprecise_dtypes=True,
                            ).then_inc(s_warm, 1)
                # small trio rides the w0b/w1 era; needed much later
                gpsimd.wait_ge(s_xa, 8)
                gpsimd.dma_start(coefT_t[:], coefT[:]).then_inc(s_gc, 16)
                gpsimd.dma_start(biasT_t[:], biasT[:]).then_inc(s_gb, 16)
                gpsimd.dma_start(coef_t[:], cv[:]).then_inc(s_gf, 16)
                gpsimd.wait_ge(s_out, 32)

            @block.tensor
            def _(pe):
                def warm(cnt):
                    for _ in range(cnt):
                        nc.tensor.matmul(pss[7][:, 0:WARMC], lhsT=warm_t[:, 0:P],
                                         rhs=warm_t[:, 0:WARMC], start=True,
                                         stop=True)

                def kmms(n, b, ks, cols=None, inc=False, pc=False):
                    c0, c1 = cols or (0, CH)
                    for k in ks:
                        nc.tensor.ldweights(xT_t[:, k, b * P:(b + 1) * P])
                        mm = nc.tensor.matmul(
                            pss[BANK[n] + b][:, c0:c1],
                            lhsT=xT_t[:, k, b * P:(b + 1) * P],
                            rhs=wts[n][:, k, c0:c1],
                            start=(k == 0),
                            stop=(k == NK - 1),
                        )
                        mm.ins.ldweights = False
                        if k == NK - 1 and inc:
                            mm.then_inc(s_pe, 1)
                        if pc:
                            mm.then_inc(s_pc, 1)

                pe.wait_ge(s_warm, 1)
                warm(NWARM)
                pe.wait_ge(s_xa, 16)
                pe.wait_ge(s_wk[0], 16)
                for k in range(0, 4):
                    for b in range(NB):
                        kmms(0, b, [k])
                pe.wait_ge(s_wk[4], 16)
                for k in range(4, NK):
                    for b in range(NB):
                        kmms(0, b, [k], inc=True)           # s_pe 1,2
                warm(FILLS[0])
                pe.wait_ge(s_wk[1], 16)
                for k in range(NK):
                    for b in range(NB):
                        kmms(1, b, [k], inc=True)           # s_pe 3,4
                # bias matmuls: b0 -> s_pe=5, b1 -> s_pe=6
                pe.wait_ge(s_gc, 16)
                pe.wait_ge(s_gb, 16)
                for b in range(NB):
                    nc.tensor.matmul(
                        pss[6][:, b * OUTL:(b + 1) * OUTL],
                        lhsT=coefT_t[:, b * P:(b + 1) * P],
                        rhs=biasT_t[:],
                        start=True, stop=True,
                    ).then_inc(s_pe, 1)
                warm(FILLS[1])
                pe.wait_ge(s_wk[2], 16)
                for k in range(NK):
                    for b in range(NB):
                        kmms(2, b, [k], inc=True)           # s_pe 7,8
                warm(FILLS[2])
                pe.wait_ge(s_dvm, 2)
                pe.wait_ge(s_wk[3], 16)
                for b in (1, 0):
                    if N3SPLIT:
                        # column-split: group A (cols 0:256) finishes 8 mms
                        # early so DVE starts while PE runs group B
                        kmms(3, b, range(0, 4), cols=(0, CH // 2))
                        kmms(3, b, range(4, NK), cols=(0, CH // 2), inc=True)
                        kmms(3, b, range(0, 4), cols=(CH // 2, CH))
                        kmms(3, b, range(4, NK), cols=(CH // 2, CH), inc=True)
                    else:
                        kmms(3, b, range(NK), inc=True)
                        mmnop = nc.tensor.matmul(
                            pss[7][:, 0:P], lhsT=warm_t[:, 0:P],
                            rhs=warm_t[:, 0:P], start=True, stop=True)
                        mmnop.then_inc(s_pe, 1)

            @block.vector
            def _(vector):
                # Clear every sem except s_warm (gpsimd owns that one so
                # its iota inc is ordered after the clear). Vector is idle
                # early; this completes well before the first DMA
                # completions (~11us) or engine incs can land.
                rest = [s_xa, s_gf, s_gc, s_gb, s_pe, s_pc, s_dvm,
                        s_dve, s_out] + s_wk
                lo = min(s.num for s in rest)
                hi = max(s.num for s in rest)
                vector.sem_clear(range(lo, hi + 1))
                vector.wait_ge(s_gf, 16)
                pe_val = {(0, 0): 1, (0, 1): 2, (1, 0): 3, (1, 1): 4,
                          (2, 0): 7, (2, 1): 8}
                for n in range(3):
                    for b in range(NB):
                        vector.wait_ge(s_pe, pe_val[(n, b)])
                        coef_b = coef_t[:, b, :].rearrange(
                            "p (one r) -> p one r", one=1)
                        mul = nc.vector.tensor_mul(
                            tmp[:],
                            pss[BANK[n] + b][:].rearrange(
                                "p (o r) -> p o r", r=RANK),
                            coef_b.to_broadcast((P, OCH, RANK)),
                        )
                        if n == 0:
                            mul.then_inc(s_dvm, 1)
                        nc.vector.tensor_reduce(
                            out_sb[b][:, n * OCH:(n + 1) * OCH],
                            tmp[:],
                            axis=mybir.AxisListType.X,
                            op=mybir.AluOpType.add,
                        )
                # n3: b1 (s_pe 9/10) then b0 (11/12), half-chunks
                for i, b in enumerate((1, 0)):
                    coef_b = coef_t[:, b, :].rearrange(
                        "p (one r) -> p one r", one=1)
                    if N3SPLIT:
                        for h in range(2):
                            vector.wait_ge(s_pe, 9 + 2 * i + h)
                            hc = CH // 2
                            oh = OCH // 2
                            nc.vector.tensor_mul(
                                tmph[:],
                                pss[BANK[3] + b][:, h * hc:(h + 1) * hc].rearrange(
                                    "p (o r) -> p o r", r=RANK),
                                coef_b.to_broadcast((P, oh, RANK)),
                            )
                            nc.vector.tensor_reduce(
                                out_sb[b][:, 3 * OCH + h * oh:3 * OCH + (h + 1) * oh],
                                tmph[:],
                                axis=mybir.AxisListType.X,
                                op=mybir.AluOpType.add,
                            )
                    else:
                        vector.wait_ge(s_pe, 9 + 2 * i)
                        nc.vector.tensor_mul(
                            tmp[:],
                            pss[BANK[3] + b][:].rearrange(
                                "p (o r) -> p o r", r=RANK),
                            coef_b.to_broadcast((P, OCH, RANK)),
                        )
                        nc.vector.tensor_reduce(
                            out_sb[b][:, 3 * OCH:4 * OCH],
                            tmp[:],
                            axis=mybir.AxisListType.X,
                            op=mybir.AluOpType.add,
                        )
                    nc.vector.tensor_add(
                        outf[b][:], out_sb[b][:],
                        pss[6][:, b * OUTL:(b + 1) * OUTL],
                    ).then_inc(s_dve, 1)

    nc.compile()
    return nc


def build_nc_tile(dt_name=DT_NAME):
    dt, _ = _DT_MAP[dt_name]
    f32 = mybir.dt.float32
    # Bacc (not raw Bass): its compile() runs generate_event_semaphores,
    # which splits multi-wait sync_info into EventSemaphore prefixes —
    # walrus accepts at most one wait per regular instruction.
    nc = bacc.Bacc("TRN2", target_bir_lowering=False, debug=False)

    xT = nc.declare_dram_parameter("xT", [IN, B], dt, isOutput=False)
    # w2[n, p, k*CH+c] = W2[k*128+p, n*CH+c]: pre-swizzled on host so each
    # SBUF partition's data is one contiguous 8KB run in DRAM (full-rate DMA).
    w2 = nc.declare_dram_parameter("w2", [NCH, P, NK * CH], dt, isOutput=False)
    coef = nc.declare_dram_parameter("coef", [B, RANK], f32, isOutput=False)
    coefT = nc.declare_dram_parameter("coefT", [RANK, B], dt, isOutput=False)
    biasT = nc.declare_dram_parameter("biasT", [RANK, OUTL], dt, isOutput=False)
    out = nc.declare_dram_parameter("out", [B, OUTL], f32, isOutput=True)

    with tile.TileContext(nc) as tc, ExitStack() as ctx:
        cpool = ctx.enter_context(tc.tile_pool(name="const", bufs=1))
        wpool = ctx.enter_context(tc.tile_pool(name="w", bufs=NCH))
        ppool = ctx.enter_context(tc.tile_pool(name="proj", bufs=6, space="PSUM"))
        bpool = ctx.enter_context(tc.tile_pool(name="biasps", bufs=2, space="PSUM"))
        spool = ctx.enter_context(tc.tile_pool(name="stage2", bufs=4))
        opool = ctx.enter_context(tc.tile_pool(name="outp", bufs=2))

        # Weight tiles for every n-chunk (issued first; n=0 split so the
        # first matmuls can start after only 256KB has landed).
        wts = [wpool.tile([P, NK, CH], dt, tag="w", name=f"wt{n}")
               for n in range(NCH)]
        w2v = w2.rearrange("n p (k c) -> n p k c", c=CH)
        nc.sync.dma_start(wts[0][:, 0:2, :], w2v[0][:, 0:2, :])
        # Full inputT, split in halves (first matmuls need only low k).
        xT_t = cpool.tile([P, NK, B], dt, tag="xT")
        xTv = xT.rearrange("(k p) b -> p k b", p=P)
        nc.sync.dma_start(xT_t[:, 0:NK // 2, :], xTv[:, 0:NK // 2, :])
        nc.sync.dma_start(wts[0][:, 2:NK, :], w2v[0][:, 2:NK, :])
        nc.sync.dma_start(xT_t[:, NK // 2:, :], xTv[:, NK // 2:, :])
        for n in range(1, NCH):
            nc.sync.dma_start(wts[n][:], w2[n].rearrange("p (k c) -> p k c", c=CH))
        coef_t = cpool.tile([P, NB, RANK], f32, tag="coef")
        nc.sync.dma_start(coef_t[:], coef.rearrange("(nb p) r -> p nb r", p=P))
        coefT_t = cpool.tile([RANK, B], dt, tag="coefT")
        nc.sync.dma_start(coefT_t[:], coefT[:])
        biasT_t = cpool.tile([RANK, OUTL], dt, tag="biasT")
        nc.sync.dma_start(biasT_t[:], biasT[:])

        # Bias term: out_bias[b,o] = sum_r coef[b,r] * bias[o,r]
        bias_ps = []
        for b in range(NB):
            bp = bpool.tile([P, OUTL], f32, tag="bias")
            nc.tensor.matmul(
                bp[:], lhsT=coefT_t[:, b * P:(b + 1) * P], rhs=biasT_t[:],
                start=True, stop=True,
            )
            bias_ps.append(bp)

        out_sb = [
            opool.tile([P, OUTL], f32, tag="osum", name=f"osum{b}")
            for b in range(NB)
        ]

        for n in range(NCH):
            pss = [
                ppool.tile([P, CH], f32, tag="proj", name=f"proj{n}_{b}")
                for b in range(NB)
            ]
            wt = wts[n]
            for k in range(NK):
                for b in range(NB):
                    nc.tensor.matmul(
                        pss[b][:],
                        lhsT=xT_t[:, k, b * P:(b + 1) * P],
                        rhs=wt[:, k, :],
                        start=(k == 0),
                        stop=(k == NK - 1),
                    )
            # Rank contraction: multiply by per-(b,r) coef, reduce over r.
            for b in range(NB):
                tmp = spool.tile([P, CH], f32, tag="tmp")
                coef_b = coef_t[:, b, :].rearrange("p (one r) -> p one r", one=1)
                nc.vector.tensor_mul(
                    tmp[:].rearrange("p (o r) -> p o r", r=RANK),
                    pss[b][:].rearrange("p (o r) -> p o r", r=RANK),
                    coef_b.to_broadcast((P, OCH, RANK)),
                )
                nc.vector.tensor_reduce(
                    out_sb[b][:, n * OCH:(n + 1) * OCH],
                    tmp[:].rearrange("p (o r) -> p o r", r=RANK),
                    axis=mybir.AxisListType.X,
                    op=mybir.AluOpType.add,
                )

        for b in range(NB):
            outf = opool.tile([P, OUTL], f32, tag="outf")
            nc.vector.tensor_add(outf[:], out_sb[b][:], bias_ps[b][:])
            nc.sync.dma_start(out[b * P:(b + 1) * P, :], outf[:])

    nc.compile()
    return nc


def prepare_in_maps(input, coef, weight, bias, dt_name=DT_NAME):
    _, npdt = _DT_MAP[dt_name]
    xT = np.ascontiguousarray(input.T).astype(npdt)          # (IN, B)
    coefT = np.ascontiguousarray(coef.T).astype(npdt)        # (RANK, B)
    coef32 = np.ascontiguousarray(coef.astype(np.float32))   # (B, RANK)
    in_maps = []
    for c in range(NCORES):
        wsh = weight[c * OUTL:(c + 1) * OUTL]                # (OUTL, IN, RANK)
        # W2[i, o*RANK+r] = wsh[o, i, r]; n-major 512-col chunks; then swizzle
        # (n, i=k*128+p, c) -> (n, p, k, c) so each partition reads one
        # contiguous 8KB run per n-chunk DMA.
        w2 = wsh.transpose(1, 0, 2).reshape(IN, OUTL * RANK)
        w2 = w2.reshape(NK, P, NCH, CH).transpose(2, 1, 0, 3)
        w2 = np.ascontiguousarray(w2.reshape(NCH, P, NK * CH)).astype(npdt)
        biasT = np.ascontiguousarray(
            bias[c * OUTL:(c + 1) * OUTL].T
        ).astype(npdt)                                       # (RANK, OUTL)
        in_maps.append({
            "xT": xT, "w2": w2, "coef": coef32,
            "coefT": coefT, "biasT": biasT,
        })
    return in_maps


def prepare_in_maps2(input, coef, weight, bias, dt_name=DT_NAME):
    _, npdt = _DT_MAP[dt_name]
    # xT2[p, k*B+b] = input[b, k*128+p] -> 4KB contiguous per partition
    xT2 = np.ascontiguousarray(
        input.T.reshape(NK, P, B).transpose(1, 0, 2).reshape(P, NK * B)
    ).astype(npdt)
    # coef2[p, nb*RANK+r] = coef[nb*128+p, r]
    coef2 = np.ascontiguousarray(
        coef.reshape(NB, P, RANK).transpose(1, 0, 2).reshape(P, NB * RANK)
    ).astype(np.float32)
    coefT = np.ascontiguousarray(coef.T).astype(npdt)        # (RANK, B)
    in_maps = []
    for c in range(NCORES):
        wsh = weight[c * OUTL:(c + 1) * OUTL]                # (OUTL, IN, RANK)
        w2 = wsh.transpose(1, 0, 2).reshape(IN, OUTL * RANK)
        w2 = w2.reshape(NK, P, NCH, CH).transpose(2, 1, 0, 3)
        w2 = np.ascontiguousarray(w2.reshape(NCH, P, NK * CH)).astype(npdt)
        biasT = np.ascontiguousarray(
            bias[c * OUTL:(c + 1) * OUTL].T
        ).astype(npdt)                                       # (RANK, OUTL)
        in_maps.append({
            "xT2": xT2, "w2": w2, "coef2": coef2,
            "coefT": coefT, "biasT": biasT,
        })
    return in_maps


_NC_CACHE = {}


def _ensure_ntff_hook():
    """The agent image's antenv lacks axon_hooks; inject it and register
    the ctypes NTFF profile hook so trace=True works under axon."""
    import types
    import antenv
    try:
        from antenv import axon_hooks  # noqa: F401
        return
    except ImportError:
        pass
    mod = types.ModuleType("antenv.axon_hooks")
    _state = {"hook": None}
    mod.set_axon_ntff_profile_hook = lambda h: _state.__setitem__("hook", h)
    mod.get_axon_ntff_profile_hook = lambda: _state["hook"]
    sys.modules["antenv.axon_hooks"] = mod
    antenv.axon_hooks = mod
    try:
        from trn_agent_boot.trn_boot import _ntff_profile_via_ctypes
        mod.set_axon_ntff_profile_hook(
            _ntff_profile_via_ctypes("/opt/axon/libaxon_pjrt.so")
        )
    except Exception:
        pass


def build_nc(dt_name=DT_NAME, impl=None):
    impl = impl or IMPL
    if impl == "raw":
        return build_nc_raw(dt_name)
    if impl == "raw2":
        return build_nc_raw2(dt_name)
    return build_nc_tile(dt_name)


def run(inputs, trace=False, dt_name=DT_NAME, impl=None, **kwargs):
    if trace:
        _ensure_ntff_hook()
    impl = impl or IMPL
    key = (dt_name, impl)
    if key not in _NC_CACHE:
        _NC_CACHE[key] = build_nc(dt_name, impl)
    nc = _NC_CACHE[key]
    prep = prepare_in_maps2 if impl == "raw2" else prepare_in_maps
    in_maps = prep(
        np.asarray(inputs["input"], dtype=np.float32),
        np.asarray(inputs["coef"], dtype=np.float32),
        np.asarray(inputs["weight"], dtype=np.float32),
        np.asarray(inputs["bias"], dtype=np.float32),
        dt_name,
    )
    br = run_bass_kernel_spmd(
        nc, in_maps, list(range(NCORES)), trace=trace, **kwargs
    )
    full = np.concatenate(
        [br.results[c]["out"] for c in range(NCORES)], axis=1
    ).astype(np.float32)
    return full, br


def kernel(**inputs):
    full, _ = run(inputs)
    return full

